# revision 15
# baseline (speedup 1.0000x reference)
"""Memristor linear layer kernel for 8 TRN2 NeuronCores.

The reference memristor crossbar computation collapses algebraically to
    out = x @ weights.T + bias
(the G_OFF offsets cancel in the pos/neg column subtraction and the k_G /
k_I scale factors cancel exactly), so the kernel computes the plain linear
layer.

Precision: harness tolerance is rel_err < 2e-2; plain bf16 operands with
fp32 PSUM accumulation and a bf16 output land at ~2.9e-3, so everything
is bf16 (half the HBM traffic of a fp32-accurate hi+lo split scheme).
The (always zero per the problem spec) bias is added on host in fp32.

Sharding: tensor-parallel over the 1024 output features -> 128 per core.
Each core receives x.T (replicated, bf16) and its W.T column shard (bf16),
pre-packed on host into the exact SBUF layout [128 partitions, k_tile,
free] so every DMA moves per-partition-contiguous rows at line rate.

Schedule (from NTFF profiling on TRN2 under axon):
- Raw bass, no TileContext: no tile-entry/exit barrier rounds.  The
  input DMAs and the PE warm-ups are additionally hoisted above the
  framework's init barrier + const memsets in the main block (they have
  no waits and touch only our tensors), so desc-gen runs straight out of
  each engine's preamble instead of idling ~1 us at the barrier.
- The NEFF wrapper's epilogue makes every engine clear a fixed ~51-sem
  slice of the 256-sem space (Tensor 53 x 115 ns = 6.1 us is the long
  pole), gated on a rendezvous of ALL engines; this ~6.6 us tail is
  immutable, so the only lever is the last engine's rendezvous arrival.
  Hence nothing waits on the out-DMA completion: its receipt lands
  ~1.3 us after issue, far inside the epilogue, and its semaphore is
  never compared so the leftover count is benign across re-executions.
- w and x are packed into one [128, 3072] bf16 region (6 KB contiguous
  per partition) and fetched as two concurrent transfers, one per HWDGE
  ring: t1 = w + x k0..3 (4 KB descriptors, sync) and t2 = x k4..7
  (2 KB descriptors, scalar).  SDMA packet cost is ~fixed per packet,
  so descriptor size sets the drain rate (~1 KB -> 120 GB/s, 2 KB ->
  230, 4 KB -> 400, shared ~400-440 GB/s across both rings); matmuls
  k0..3 gate on t1 only, k4..7 on t2.
- PE garbage warm-up matmuls build toward the HAM clock-gate release
  (1.2 -> 2.4 GHz after ~3.4 us sustained busy) while the inputs stream.
"""

import os

import numpy as np

BATCH = 256
SIZE_IN = 1024
SIZE_OUT = 1024
N_CORES = 8
O_SHARD = SIZE_OUT // N_CORES  # 128
K_TILES = SIZE_IN // 128  # 8

_STATE = {}


def _build():
    import contextlib

    import concourse.bass as bass  # noqa: F401
    from concourse import bacc, mybir

    f32 = mybir.dt.float32
    bf16 = mybir.dt.bfloat16

    n_warm_big = int(os.environ.get("WARM_BIG", "5"))
    n_warm_small = int(os.environ.get("WARM_SMALL", "6"))
    n_warm_post = int(os.environ.get("WARM_POST", "0"))
    out_split = os.environ.get("OUT_SPLIT", "0") == "1"
    final_wait = os.environ.get("FINAL_WAIT", "0") == "1"
    hoist_dma = os.environ.get("HOIST_DMA", "1") == "1"
    hoist_pe = os.environ.get("HOIST_PE", "1") == "1"

    nc = bacc.Bacc(None, target_bir_lowering=False)

    # w and x ride ONE transfer: [128, 3072] bf16 = 6 KB contiguous per
    # partition (cols 0:1024 = w packed [kt, of], cols 1024:3072 = x packed
    # [kt, batch]).  Descriptor size = per-partition contiguous run, and the
    # SDMA packet cost is ~fixed per packet, so 6 KB descriptors drain at
    # ~2x the rate of separate 2 KB w descriptors.
    wx_d = nc.declare_dram_parameter("wx", [128, 3072], bf16, isOutput=False)
    out_d = nc.declare_dram_parameter("out", [O_SHARD, BATCH], bf16, isOutput=True)

    with contextlib.ExitStack() as stack:
        wx_s = stack.enter_context(nc.sbuf_tensor([128, 3072], bf16))
        o_s = stack.enter_context(nc.sbuf_tensor([O_SHARD, BATCH], bf16))
        warm_s = stack.enter_context(nc.sbuf_tensor([128, 512], bf16))
        pt = stack.enter_context(nc.psum_tensor([O_SHARD, BATCH], f32))
        warm_pt = stack.enter_context(nc.psum_tensor([128, 512], f32))
        s_wx = stack.enter_context(nc.semaphore("s_wx"))
        s_mm = stack.enter_context(nc.semaphore("s_mm"))
        s_c0 = stack.enter_context(nc.semaphore("s_c0"))
        s_c1 = stack.enter_context(nc.semaphore("s_c1"))
        s_o = stack.enter_context(nc.semaphore("s_o"))

        # Two concurrent 3 KB/partition transfers, one per HWDGE ring:
        # t1 = w + x k-tiles 0,1 (sync), t2 = x k-tiles 2..7 (scalar).
        # Both rings share ~400 GB/s; 256 descriptors spread the SDMA
        # engine load better than 128 (the last descs of a single big
        # transfer crawl), and matmuls k0,k1 can start on t1 alone.
        split = int(os.environ.get("WX_SPLIT", "2048"))
        dma_in = (
            nc.sync.dma_start(out=wx_s[:, 0:split], in_=wx_d[:, 0:split])
            .then_inc(s_wx, 16)
        )
        dma_in2 = (
            nc.scalar.dma_start(out=wx_s[:, split:], in_=wx_d[:, split:])
            .then_inc(s_mm2 := stack.enter_context(nc.semaphore("s_t2")), 16)
        )

        # PE warm-up on garbage SBUF into a scratch PSUM bank.
        warms = []
        for _ in range(n_warm_big):
            warms.append(
                nc.tensor.matmul(
                    warm_pt[:], warm_s[:, 0:128], warm_s[:], start=True, stop=True
                )
            )
        for _ in range(n_warm_small):
            nc.tensor.matmul(
                warm_pt[:, 0:64], warm_s[:, 0:128], warm_s[:, 0:64],
                start=True, stop=True,
            )

        kt_split = max(0, (split - 1024) // BATCH)  # k-tiles covered by t1
        nc.tensor.wait_ge(s_wx, 16)
        for k in range(K_TILES):
            if k == kt_split:
                nc.tensor.wait_ge(s_mm2, 16)
            mm = nc.tensor.matmul(
                pt[:],
                wx_s[:, k * 128 : (k + 1) * 128],
                wx_s[:, 1024 + k * BATCH : 1024 + (k + 1) * BATCH],
                start=(k == 0),
                stop=(k == K_TILES - 1),
            )
            if k == K_TILES - 1:
                mm.then_inc(s_mm, 1)
        # Post-work garbage matmuls keep the PE busy until the epilogue
        # rendezvous so the HAM boost grace window covers the Tensor
        # engine's epilogue sem-clear loop.
        for _ in range(n_warm_post):
            nc.tensor.matmul(
                warm_pt[:, 0:64], warm_s[:, 0:128], warm_s[:, 0:64],
                start=True, stop=True,
            )

        # PSUM -> SBUF with fp32 -> bf16 cast, split in batch halves so the
        # two output DMAs' desc-gen (SP and ACT rings) overlap the casts.
        hb = BATCH // 2
        nc.vector.wait_ge(s_mm, 1)
        if out_split:
            nc.vector.tensor_copy(out=o_s[:, 0:hb], in_=pt[:, 0:hb]).then_inc(s_c0, 1)
            nc.sync.wait_ge(s_c0, 1)
            nc.sync.dma_start(out=out_d[:, 0:hb], in_=o_s[:, 0:hb]).then_inc(s_o, 16)
            nc.vector.tensor_copy(out=o_s[:, hb:], in_=pt[:, hb:]).then_inc(s_c1, 1)
            nc.scalar.wait_ge(s_c1, 1)
            nc.scalar.dma_start(out=out_d[:, hb:], in_=o_s[:, hb:]).then_inc(s_o, 16)
        else:
            nc.vector.tensor_copy(out=o_s[:], in_=pt[:]).then_inc(s_c0, 1)
            nc.sync.wait_ge(s_c0, 1)
            nc.sync.dma_start(out=out_d[:], in_=o_s[:]).then_inc(s_o, 32)

        if final_wait:
            nc.sync.wait_ge(s_o, 32)
            nums = sorted(s.num for s in (s_wx, s_mm, s_c0, s_c1, s_o))
            nc.sync.sem_clear(range(nums[0], nums[-1] + 1))
        # else: the NEFF wrapper's epilogue clears every semaphore anyway;
        # s_o may be cleared before the out receipt increments it, but its
        # value is never compared, so the leftover count is harmless.

        # Hoist the input DMA desc-gen (SP) and the PE warm-ups above the
        # framework's init barrier + const memsets in the main block: they
        # have no waits and touch only our tensors, so each engine can run
        # them straight out of its preamble instead of idling ~1.1 us at
        # the barrier behind the (unrelated) const-tile memsets.
        hoisted = []
        if hoist_dma:
            hoisted.append(dma_in.ins)
            hoisted.append(dma_in2.ins)
        if hoist_pe:
            hoisted.extend(w.ins for w in warms)
        if hoisted:
            blk = nc.m.functions[0].blocks[0]
            insts = list(blk.instructions)
            assert insts[0].__class__.__name__ == "InstCall", insts[0]
            names = [h.name for h in hoisted]
            nameset = set(names)
            assert len(nameset) == len(names)
            by_name = {i.name: i for i in insts}
            rest = [i for i in insts[1:] if i.name not in nameset]
            assert len(rest) + len(names) + 1 == len(insts)
            blk.instructions = (
                [insts[0]] + [by_name[n] for n in names] + rest
            )

        nc.compile()
    return nc


def _install_ntff_hook_shim():
    """The agent image's antenv lacks axon_hooks; recreate it so
    run_bass_kernel_spmd(trace=True) can capture NTFF profiles."""
    import sys
    import types

    if "antenv.axon_hooks" in sys.modules:
        return
    try:
        import antenv.axon_hooks  # noqa: F401  (real module exists)

        return
    except ImportError:
        pass
    mod = types.ModuleType("antenv.axon_hooks")
    mod._HOOK = None

    def set_axon_ntff_profile_hook(hook):
        mod._HOOK = hook

    def get_axon_ntff_profile_hook():
        return mod._HOOK

    mod.set_axon_ntff_profile_hook = set_axon_ntff_profile_hook
    mod.get_axon_ntff_profile_hook = get_axon_ntff_profile_hook
    sys.modules["antenv.axon_hooks"] = mod
    try:
        from trn_agent_boot.trn_boot import _ntff_profile_via_ctypes

        mod._HOOK = _ntff_profile_via_ctypes("/opt/axon/libaxon_pjrt.so")
    except Exception:
        pass


def _pack(a_t: np.ndarray, ncols: int) -> np.ndarray:
    """[SIZE_IN, ncols] f32 -> bf16 packed as [128, K_TILES, ncols]."""
    import ml_dtypes

    return np.ascontiguousarray(
        a_t.astype(ml_dtypes.bfloat16).reshape(K_TILES, 128, ncols).transpose(1, 0, 2)
    )


def kernel(x: np.ndarray, weights: np.ndarray, bias: np.ndarray) -> np.ndarray:
    from concourse.bass_utils import run_bass_kernel_spmd

    if "nc" not in _STATE:
        _STATE["nc"] = _build()
    nc = _STATE["nc"]

    x = np.asarray(x, dtype=np.float32)
    weights = np.asarray(weights, dtype=np.float32)
    bias = np.asarray(bias, dtype=np.float32)

    xt = _pack(np.ascontiguousarray(x.T), BATCH)  # [128, K_TILES, BATCH] bf16
    xt2 = xt.reshape(128, K_TILES * BATCH)
    wt = np.ascontiguousarray(weights.T)  # [SIZE_IN, SIZE_OUT] f32

    in_maps = []
    for c in range(N_CORES):
        sl = slice(c * O_SHARD, (c + 1) * O_SHARD)
        wp = _pack(np.ascontiguousarray(wt[:, sl]), O_SHARD)
        wx = np.concatenate([wp.reshape(128, K_TILES * O_SHARD), xt2], axis=1)
        in_maps.append({"wx": np.ascontiguousarray(wx)})

    # Always install the shim: if BASS_TRACE is set in the environment,
    # run_bass_kernel_spmd imports antenv.axon_hooks unconditionally and
    # would otherwise crash on images whose antenv lacks that module.
    _install_ntff_hook_shim()
    trace = os.environ.get("BASS_PROBLEM_TRACE", "0") == "1"
    res = run_bass_kernel_spmd(
        nc, in_maps, core_ids=list(range(N_CORES)), trace=trace
    )
    _STATE["last_results"] = res

    out_t = np.concatenate(
        [np.asarray(res.results[c]["out"]) for c in range(N_CORES)], axis=0
    )  # [SIZE_OUT, BATCH] bf16
    out = out_t.T.astype(np.float32) + bias[None, :]
    return np.ascontiguousarray(out)


# revision 17
# speedup vs baseline: 1.0727x; 1.0727x over previous
"""Memristor linear layer kernel for 8 TRN2 NeuronCores.

The reference memristor crossbar computation collapses algebraically to
    out = x @ weights.T + bias
(the G_OFF offsets cancel in the pos/neg column subtraction and the k_G /
k_I scale factors cancel exactly), so the kernel computes the plain linear
layer.

Precision: harness tolerance is rel_err < 2e-2; plain bf16 operands with
fp32 PSUM accumulation and a bf16 output land at ~2.9e-3, so everything
is bf16 (half the HBM traffic of a fp32-accurate hi+lo split scheme).
The (always zero per the problem spec) bias is added on host in fp32.

Sharding: tensor-parallel over the 1024 output features -> 128 per core.
Each core receives x.T (replicated, bf16) and its W.T column shard (bf16),
pre-packed on host into the exact SBUF layout [128 partitions, k_tile,
free] so every DMA moves per-partition-contiguous rows at line rate.

Schedule (from NTFF profiling on TRN2 under axon):
- Raw bass, no TileContext: no tile-entry/exit barrier rounds.  The
  input DMAs and the PE warm-ups are additionally hoisted above the
  framework's init barrier + const memsets in the main block (they have
  no waits and touch only our tensors), so desc-gen runs straight out of
  each engine's preamble instead of idling ~1 us at the barrier.
- The NEFF wrapper's epilogue makes every engine clear a fixed ~51-sem
  slice of the 256-sem space (Tensor 53 x 115 ns = 6.1 us is the long
  pole), gated on a rendezvous of ALL engines; this ~6.6 us tail is
  immutable, so the only lever is the last engine's rendezvous arrival.
  Hence nothing waits on the out-DMA completion: its receipt lands
  ~1.3 us after issue, far inside the epilogue, and its semaphore is
  never compared so the leftover count is benign across re-executions.
- w and x are packed into one [128, 3072] bf16 region (6 KB contiguous
  per partition) and fetched as two concurrent transfers, one per HWDGE
  ring: t1 = w + x k0..3 (4 KB descriptors, sync) and t2 = x k4..7
  (2 KB descriptors, scalar).  SDMA packet cost is ~fixed per packet,
  so descriptor size sets the drain rate (~1 KB -> 120 GB/s, 2 KB ->
  230, 4 KB -> 400, shared ~400-440 GB/s across both rings); matmuls
  k0..3 gate on t1 only, k4..7 on t2.
- PE garbage warm-up matmuls build toward the HAM clock-gate release
  (1.2 -> 2.4 GHz after ~3.4 us sustained busy) while the inputs stream.
"""

import os

import numpy as np

BATCH = 256
SIZE_IN = 1024
SIZE_OUT = 1024
N_CORES = 8
O_SHARD = SIZE_OUT // N_CORES  # 128
K_TILES = SIZE_IN // 128  # 8

_STATE = {}


def _build():
    import contextlib

    import concourse.bass as bass  # noqa: F401
    from concourse import bacc, mybir

    f32 = mybir.dt.float32
    bf16 = mybir.dt.bfloat16

    n_warm_big = int(os.environ.get("WARM_BIG", "3"))
    n_warm_small = int(os.environ.get("WARM_SMALL", "3"))
    n_warm_post = int(os.environ.get("WARM_POST", "0"))
    out_split = os.environ.get("OUT_SPLIT", "0") == "1"
    final_wait = os.environ.get("FINAL_WAIT", "0") == "1"
    hoist_dma = os.environ.get("HOIST_DMA", "1") == "1"
    hoist_pe = os.environ.get("HOIST_PE", "1") == "1"

    nc = bacc.Bacc(None, target_bir_lowering=False)

    # w and x ride ONE transfer: [128, 3072] bf16 = 6 KB contiguous per
    # partition (cols 0:1024 = w packed [kt, of], cols 1024:3072 = x packed
    # [kt, batch]).  Descriptor size = per-partition contiguous run, and the
    # SDMA packet cost is ~fixed per packet, so 6 KB descriptors drain at
    # ~2x the rate of separate 2 KB w descriptors.
    wx_d = nc.declare_dram_parameter("wx", [128, 3072], bf16, isOutput=False)
    out_d = nc.declare_dram_parameter("out", [O_SHARD, BATCH], bf16, isOutput=True)

    with contextlib.ExitStack() as stack:
        wx_s = stack.enter_context(nc.sbuf_tensor([128, 3072], bf16))
        o_s = stack.enter_context(nc.sbuf_tensor([O_SHARD, BATCH], bf16))
        warm_s = stack.enter_context(nc.sbuf_tensor([128, 512], bf16))
        pt = stack.enter_context(nc.psum_tensor([O_SHARD, BATCH], f32))
        warm_pt = stack.enter_context(nc.psum_tensor([128, 512], f32))
        s_wx = stack.enter_context(nc.semaphore("s_wx"))
        s_mm = stack.enter_context(nc.semaphore("s_mm"))
        s_c0 = stack.enter_context(nc.semaphore("s_c0"))
        s_c1 = stack.enter_context(nc.semaphore("s_c1"))
        s_o = stack.enter_context(nc.semaphore("s_o"))

        # Two transfers, both on the scalar (ACT) engine: its preamble is
        # ~0.55 us shorter than sync's (no 705 ns DRAIN before the first
        # desc-gen), and one ring drains strictly FIFO — no cross-ring
        # arbitration variance.  t1 = w + x k0..3 (4 KB descriptors),
        # t2 = x k4..7 (2 KB descriptors); matmuls k0..3 gate on t1 so
        # they overlap t2's drain.
        split = int(os.environ.get("WX_SPLIT", "2048"))
        dma_in = (
            nc.scalar.dma_start(out=wx_s[:, 0:split], in_=wx_d[:, 0:split])
            .then_inc(s_wx, 16)
        )
        dma_in2 = (
            nc.scalar.dma_start(out=wx_s[:, split:], in_=wx_d[:, split:])
            .then_inc(s_mm2 := stack.enter_context(nc.semaphore("s_t2")), 16)
        )

        # PE warm-up on garbage SBUF into a scratch PSUM bank.
        warms = []
        for _ in range(n_warm_big):
            warms.append(
                nc.tensor.matmul(
                    warm_pt[:], warm_s[:, 0:128], warm_s[:], start=True, stop=True
                )
            )
        for _ in range(n_warm_small):
            nc.tensor.matmul(
                warm_pt[:, 0:64], warm_s[:, 0:128], warm_s[:, 0:64],
                start=True, stop=True,
            )

        kt_split = max(0, (split - 1024) // BATCH)  # k-tiles covered by t1
        nc.tensor.wait_ge(s_wx, 16)
        for k in range(K_TILES):
            if k == kt_split:
                nc.tensor.wait_ge(s_mm2, 16)
            mm = nc.tensor.matmul(
                pt[:],
                wx_s[:, k * 128 : (k + 1) * 128],
                wx_s[:, 1024 + k * BATCH : 1024 + (k + 1) * BATCH],
                start=(k == 0),
                stop=(k == K_TILES - 1),
            )
            if k == K_TILES - 1:
                mm.then_inc(s_mm, 1)
        # Post-work garbage matmuls keep the PE busy until the epilogue
        # rendezvous so the HAM boost grace window covers the Tensor
        # engine's epilogue sem-clear loop.
        for _ in range(n_warm_post):
            nc.tensor.matmul(
                warm_pt[:, 0:64], warm_s[:, 0:128], warm_s[:, 0:64],
                start=True, stop=True,
            )

        # PSUM -> SBUF with fp32 -> bf16 cast, split in batch halves so the
        # two output DMAs' desc-gen (SP and ACT rings) overlap the casts.
        hb = BATCH // 2
        nc.vector.wait_ge(s_mm, 1)
        if out_split:
            nc.vector.tensor_copy(out=o_s[:, 0:hb], in_=pt[:, 0:hb]).then_inc(s_c0, 1)
            nc.sync.wait_ge(s_c0, 1)
            nc.sync.dma_start(out=out_d[:, 0:hb], in_=o_s[:, 0:hb]).then_inc(s_o, 16)
            nc.vector.tensor_copy(out=o_s[:, hb:], in_=pt[:, hb:]).then_inc(s_c1, 1)
            nc.scalar.wait_ge(s_c1, 1)
            nc.scalar.dma_start(out=out_d[:, hb:], in_=o_s[:, hb:]).then_inc(s_o, 16)
        else:
            nc.vector.tensor_copy(out=o_s[:], in_=pt[:]).then_inc(s_c0, 1)
            nc.sync.wait_ge(s_c0, 1)
            nc.sync.dma_start(out=out_d[:], in_=o_s[:]).then_inc(s_o, 32)

        if final_wait:
            nc.sync.wait_ge(s_o, 32)
            nums = sorted(s.num for s in (s_wx, s_mm, s_c0, s_c1, s_o))
            nc.sync.sem_clear(range(nums[0], nums[-1] + 1))
        # else: the NEFF wrapper's epilogue clears every semaphore anyway;
        # s_o may be cleared before the out receipt increments it, but its
        # value is never compared, so the leftover count is harmless.

        # Hoist the input DMA desc-gen (SP) and the PE warm-ups above the
        # framework's init barrier + const memsets in the main block: they
        # have no waits and touch only our tensors, so each engine can run
        # them straight out of its preamble instead of idling ~1.1 us at
        # the barrier behind the (unrelated) const-tile memsets.
        hoisted = []
        if hoist_dma:
            hoisted.append(dma_in.ins)
            hoisted.append(dma_in2.ins)
        if hoist_pe:
            hoisted.extend(w.ins for w in warms)
        if hoisted:
            blk = nc.m.functions[0].blocks[0]
            insts = list(blk.instructions)
            assert insts[0].__class__.__name__ == "InstCall", insts[0]
            names = [h.name for h in hoisted]
            nameset = set(names)
            assert len(nameset) == len(names)
            by_name = {i.name: i for i in insts}
            rest = [i for i in insts[1:] if i.name not in nameset]
            assert len(rest) + len(names) + 1 == len(insts)
            blk.instructions = (
                [insts[0]] + [by_name[n] for n in names] + rest
            )

        nc.compile()
    return nc


def _install_ntff_hook_shim():
    """The agent image's antenv lacks axon_hooks; recreate it so
    run_bass_kernel_spmd(trace=True) can capture NTFF profiles."""
    import sys
    import types

    if "antenv.axon_hooks" in sys.modules:
        return
    try:
        import antenv.axon_hooks  # noqa: F401  (real module exists)

        return
    except ImportError:
        pass
    mod = types.ModuleType("antenv.axon_hooks")
    mod._HOOK = None

    def set_axon_ntff_profile_hook(hook):
        mod._HOOK = hook

    def get_axon_ntff_profile_hook():
        return mod._HOOK

    mod.set_axon_ntff_profile_hook = set_axon_ntff_profile_hook
    mod.get_axon_ntff_profile_hook = get_axon_ntff_profile_hook
    sys.modules["antenv.axon_hooks"] = mod
    try:
        from trn_agent_boot.trn_boot import _ntff_profile_via_ctypes

        mod._HOOK = _ntff_profile_via_ctypes("/opt/axon/libaxon_pjrt.so")
    except Exception:
        pass


def _pack(a_t: np.ndarray, ncols: int) -> np.ndarray:
    """[SIZE_IN, ncols] f32 -> bf16 packed as [128, K_TILES, ncols]."""
    import ml_dtypes

    return np.ascontiguousarray(
        a_t.astype(ml_dtypes.bfloat16).reshape(K_TILES, 128, ncols).transpose(1, 0, 2)
    )


def kernel(x: np.ndarray, weights: np.ndarray, bias: np.ndarray) -> np.ndarray:
    from concourse.bass_utils import run_bass_kernel_spmd

    if "nc" not in _STATE:
        _STATE["nc"] = _build()
    nc = _STATE["nc"]

    x = np.asarray(x, dtype=np.float32)
    weights = np.asarray(weights, dtype=np.float32)
    bias = np.asarray(bias, dtype=np.float32)

    xt = _pack(np.ascontiguousarray(x.T), BATCH)  # [128, K_TILES, BATCH] bf16
    xt2 = xt.reshape(128, K_TILES * BATCH)
    wt = np.ascontiguousarray(weights.T)  # [SIZE_IN, SIZE_OUT] f32

    in_maps = []
    for c in range(N_CORES):
        sl = slice(c * O_SHARD, (c + 1) * O_SHARD)
        wp = _pack(np.ascontiguousarray(wt[:, sl]), O_SHARD)
        wx = np.concatenate([wp.reshape(128, K_TILES * O_SHARD), xt2], axis=1)
        in_maps.append({"wx": np.ascontiguousarray(wx)})

    # Always install the shim: if BASS_TRACE is set in the environment,
    # run_bass_kernel_spmd imports antenv.axon_hooks unconditionally and
    # would otherwise crash on images whose antenv lacks that module.
    _install_ntff_hook_shim()
    trace = os.environ.get("BASS_PROBLEM_TRACE", "0") == "1"
    res = run_bass_kernel_spmd(
        nc, in_maps, core_ids=list(range(N_CORES)), trace=trace
    )
    _STATE["last_results"] = res

    out_t = np.concatenate(
        [np.asarray(res.results[c]["out"]) for c in range(N_CORES)], axis=0
    )  # [SIZE_OUT, BATCH] bf16
    out = out_t.T.astype(np.float32) + bias[None, :]
    return np.ascontiguousarray(out)
